# revision 57
# baseline (speedup 1.0000x reference)
"""Trainium2 Bass kernel for nn_Kernel_Conv (conv3x3+GELU -> per-pixel 19x19
conv -> conv3x3+sigmoid), SPMD over 8 NeuronCores.

Sharding: 8 cores = 2 batches x 4 H-slices (32 output rows each). All inputs
are host-preprocessed into per-core slabs (bf16) so the device program is
identical on every core.

Per-pixel conv strategy (v2, group-blocked): for output row y, split the 128
x-columns into 8 groups of 16. For group g, the contribution to out[c, w]
for w in the 34-wide window [16g-9, 16g+25) is a matmul contracting over
(kh, v): lhsT[(kh,v), c] = x[y+kh-9, 16g+v, c] (an on-chip replicated
transpose of x) and rhs[(kh,v), j] = ker[kh*19 + (v-j+18), y, 16g-9+j]
(host-skewed compact band, zero outside the valid diagonal). kh is packed in
octets of 8 (8*16 = 128 contraction partitions), so each (y, g) takes 3
matmuls (kh 0-7, 8-15, 16-18) accumulating into a per-y psum row window.
Overlapping windows accumulate onto a DVE-zeroed psum (start=False).

The band reaches SBUF via contiguous >=4KB-per-partition DMA descriptors (the
host bakes the skew + zeros), eliminating the 68-byte scatter packets that
made the previous version DMA-bound.
"""

import sys
import types

for _p in ("/opt/trn_rl_repo",):
    if _p not in sys.path:
        sys.path.insert(0, _p)

import numpy as np
import ml_dtypes
from contextlib import ExitStack

# Register the NTFF profile hook shim (harmless if tracing is never used)
try:
    import antenv  # noqa: F401
    if "antenv.axon_hooks" not in sys.modules:
        if "/root/.axon_site" not in sys.path:
            sys.path.insert(0, "/root/.axon_site")
        from trn_agent_boot.trn_boot import _ntff_profile_via_ctypes
        _hook = _ntff_profile_via_ctypes("/opt/axon/libaxon_pjrt.so")
        _mod = types.ModuleType("antenv.axon_hooks")
        _mod.get_axon_ntff_profile_hook = lambda: _hook
        sys.modules["antenv.axon_hooks"] = _mod
        antenv.axon_hooks = _mod
except Exception:
    pass

import bass_rust
import concourse.bass as bass
import concourse.tile as tile
from concourse import bacc, mybir
from concourse.bass_utils import run_bass_kernel_spmd

BF16 = np.float16

# ---------------- problem constants (hardcoded per the harness contract) ----
B, C, H, W = 2, 16, 128, 128
KK = 19            # per-pixel kernel size
NCORES = 8
HS = 32            # output rows per core
NY = 36            # y rows per core: [h0-2, h0+34)
NX = 56            # x (conv1 out) rows per core: [h0-11, h0+45)
NIN = 58           # input rows per core: [h0-12, h0+46)
G = 16             # x-column group size for the per-pixel matmuls
NG = W // G        # 8 groups
WIN = G + KK - 1   # 34: output-column window per group
NBLK = NY // 4     # 9 per-pixel blocks of 4 y rows
MROW = 2 * NG * 4 * WIN       # 2176: main band els per (partition, block)
TROW = NG * 4 * WIN           # 1088: tail band els per (partition, block)
XSLAB = NY * 16               # 576: (y, c) els per xrep slab


def _host_prepare(input, kernel, w1, b1, w2, b2):
    """Build the per-core input maps (all numpy, bf16 except biases/ident)."""
    inp = np.asarray(input, np.float32)
    ker = np.asarray(kernel, np.float32)

    # input, zero-padded: rows [-12, 140), cols [-1, 129)
    inp_pad = np.zeros((B, C, H + 27, W + 2), np.float32)
    inp_pad[:, :, 12:12 + H, 1:1 + W] = inp

    # conv weights as lhsT[(dx,c), o] per dy: [3, 48, 16]
    def wstack(wmat, order=(0, 1, 2)):
        ws = np.zeros((3, 48, 16), np.float32)
        for dy in range(3):
            for gi, dx in enumerate(order):
                ws[dy, gi * 16:gi * 16 + 16, :] = wmat[:, :, dy, dx].T
        return np.ascontiguousarray(ws.transpose(1, 0, 2)).astype(BF16)

    w1s = wstack(np.asarray(w1, np.float32))
    # stacked lhsT for the packed conv1: [(dy{0,1}, dx, c), o] = [96, 16]
    w1s6 = np.ascontiguousarray(
        w1s.reshape(48, 3, 16)[:, 0:2, :].transpose(1, 0, 2).reshape(96, 16))
    w2s = wstack(np.asarray(w2, np.float32), order=(1, 0, 2))
    # per-tap lhsT for the final conv2 block: [c', (dy, dx, o)]
    w2tap = np.ascontiguousarray(
        np.asarray(w2, np.float32).transpose(1, 2, 3, 0).reshape(16, 144)
    ).astype(BF16)
    b1t = np.asarray(b1, np.float32).reshape(16, 1).copy()
    b2t = np.asarray(b2, np.float32).reshape(16, 1).copy()
    ident = np.eye(16, dtype=BF16)

    # ---- compact skewed band ---------------------------------------------
    # main: F[(i8,v16), blk9, ys4, g8, o2, j34], kh = 8o+i
    # tail: T[(i3,v16), blk9, ys4, g8, j34],     kh = 16+i
    # value = ker[b, kh*19 + (v-j+18), y, 16g-9+j] masked to validity, where
    # y = h0-2+4blk+ys; mask: kw in [0,19), w in [0,W), y in [0,H),
    # x-row y+kh-9 in [0,H).
    def band(kh_arr, h0, b):
        # kh_arr: [NI] kernel-row indices; returns [NI*16, NBLK*rest]
        NI = kh_arr.shape[0]
        i_ = kh_arr.reshape(NI, 1, 1, 1, 1, 1, 1)
        v_ = np.arange(G).reshape(1, G, 1, 1, 1, 1, 1)
        bl = np.arange(NBLK).reshape(1, 1, NBLK, 1, 1, 1, 1)
        ys = np.arange(4).reshape(1, 1, 1, 4, 1, 1, 1)
        g_ = np.arange(NG).reshape(1, 1, 1, 1, NG, 1, 1)
        j_ = np.arange(WIN).reshape(1, 1, 1, 1, 1, 1, WIN)
        kh = i_
        y = h0 - 2 + 4 * bl + ys
        kw = v_ - j_ + 18
        w = G * g_ - 9 + j_
        xr = y + kh - 9
        valid = ((kw >= 0) & (kw < KK) & (w >= 0) & (w < W)
                 & (y >= 0) & (y < H) & (xr >= 0) & (xr < H))
        kwc = np.clip(kw, 0, KK - 1)
        wc = np.clip(w, 0, W - 1)
        yc = np.clip(y, 0, H - 1)
        F = ker[b, kh * KK + kwc, yc, wc] * valid
        # [NI, v, blk, ys, g, 1, j] -> [NI*16, blk*(ys*g*1*j)]
        F = np.broadcast_to(F, (NI, G, NBLK, 4, NG, 1, WIN))
        return np.ascontiguousarray(
            F.reshape(NI * G, NBLK * 4 * NG * WIN)).astype(BF16)

    in_maps = []
    for cid in range(NCORES):
        b = cid // 4
        h0 = 32 * (cid % 4)

        # dx-tripled input slab, doubled for dy-packing: partitions 0-47 =
        # (dx, c) at row r, partitions 48-95 = same but at row r+1 (so conv1
        # taps dy=0,1 contract in ONE 96-partition matmul)
        inp6 = np.zeros((96, NIN + 1, W), np.float32)
        rows = inp_pad[b, :, 12 + h0 - 12: 12 + h0 - 12 + NIN + 1, :]
        for dx in range(3):
            inp6[dx * 16:dx * 16 + 16, :, :] = rows[:, :, dx:dx + W]
        inp6[48:96, :NIN, :] = inp6[0:48, 1:NIN + 1, :]
        inp3 = inp6[:, :NIN, :].astype(BF16)

        # main band: interleave octets o=0,1 -> [(i,v), blk, ys, g, o, j]
        bm0 = band(np.arange(0, 8), h0, b).reshape(128, NBLK, 4, NG, 1, WIN)
        bm1 = band(np.arange(8, 16), h0, b).reshape(128, NBLK, 4, NG, 1, WIN)
        bandM = np.concatenate([bm0, bm1], axis=4)  # o dim
        bandM = np.ascontiguousarray(
            bandM.reshape(128, NBLK * MROW)).astype(BF16)
        bandT = band(np.arange(16, 19), h0, b)      # [48, NBLK*TROW]

        in_maps.append({
            "inp3": np.ascontiguousarray(inp3.reshape(96, NIN * W)),
            "bandM": bandM,
            "bandT": bandT,
            "w1s": np.ascontiguousarray(w1s.reshape(48, 3 * 16)),
            "w1s6": w1s6,
            "w2s": np.ascontiguousarray(w2s.reshape(48, 3 * 16)),
            "w2tap": w2tap,
            "b1t": b1t,
            "b2t": b2t,
            "ident": ident,
        })
    return in_maps


DEBUG_DUMP = False


def _build_program():
    nc = bacc.Bacc("TRN2", target_bir_lowering=False, debug=False,
                   num_devices=NCORES)
    dt = mybir.dt

    inp3_d = nc.dram_tensor("inp3", [96, NIN * W], dt.float16,
                            kind="ExternalInput").ap()
    bandM_d = nc.dram_tensor("bandM", [128, NBLK * MROW], dt.float16,
                             kind="ExternalInput").ap()
    bandT_d = nc.dram_tensor("bandT", [48, NBLK * TROW], dt.float16,
                             kind="ExternalInput").ap()
    w1s_d = nc.dram_tensor("w1s", [48, 3 * 16], dt.float16,
                           kind="ExternalInput").ap()
    w1s6_d = nc.dram_tensor("w1s6", [96, 16], dt.float16,
                            kind="ExternalInput").ap()
    w2s_d = nc.dram_tensor("w2s", [48, 3 * 16], dt.float16,
                           kind="ExternalInput").ap()
    w2tap_d = nc.dram_tensor("w2tap", [16, 9 * 16], dt.float16,
                             kind="ExternalInput").ap()
    b1t_d = nc.dram_tensor("b1t", [16, 1], dt.float32, kind="ExternalInput").ap()
    b2t_d = nc.dram_tensor("b2t", [16, 1], dt.float32, kind="ExternalInput").ap()
    ident_d = nc.dram_tensor("ident", [16, 16], dt.float16,
                             kind="ExternalInput").ap()
    out_d = nc.dram_tensor("out", [16, HS * W], dt.float32,
                           kind="ExternalOutput").ap()
    xTd_d = nc.dram_tensor("xTd", [128, NX * 16], dt.float16,
                           kind="Internal").ap()
    dbg = None
    if DEBUG_DUMP:
        dbg = {
            "xT": nc.dram_tensor("dbg_xT", [128, NX * 16], dt.float16,
                                 kind="ExternalOutput").ap(),
            "xrepM": nc.dram_tensor("dbg_xrepM", [128, NG * 2 * XSLAB],
                                    dt.float16, kind="ExternalOutput").ap(),
            "xrepT": nc.dram_tensor("dbg_xrepT", [48, NG * XSLAB],
                                    dt.float16, kind="ExternalOutput").ap(),
            "y3": nc.dram_tensor("dbg_y3", [48, NY * W], dt.float16,
                                 kind="ExternalOutput").ap(),
        }

    with tile.TileContext(nc) as tc:
        with ExitStack() as ctx:
            _body(ctx, tc, inp3_d, bandM_d, bandT_d, w1s_d, w1s6_d, w2s_d,
                  w2tap_d, b1t_d, b2t_d, ident_d, out_d, xTd_d, dbg)
    nc.compile()
    return nc


def _body(ctx, tc, inp3_d, bandM_d, bandT_d, w1s_d, w1s6_d, w2s_d, w2tap_d,
          b1t_d, b2t_d, ident_d, out_d, xTd_d, dbg=None):
    nc = tc.nc
    dt = mybir.dt
    AFT = mybir.ActivationFunctionType

    consts = ctx.enter_context(tc.tile_pool(name="consts", bufs=1))
    bigs = ctx.enter_context(tc.tile_pool(name="bigs", bufs=1))
    ps_c1 = ctx.enter_context(tc.tile_pool(name="ps_c1", bufs=2, space="PSUM"))
    ps_tp = ctx.enter_context(tc.tile_pool(name="ps_tp", bufs=2, space="PSUM"))
    ps_pp = ctx.enter_context(tc.tile_pool(name="ps_pp", bufs=2, space="PSUM"))
    ps_c2 = ctx.enter_context(tc.tile_pool(name="ps_c2", bufs=2, space="PSUM"))

    # ---- persistent SBUF tiles -------------------------------------------
    w1s_t = consts.tile([48, 3 * 16], dt.float16, tag="w1s")
    w1s6_t = consts.tile([96, 16], dt.float16, tag="w1s6")
    w2s_t = consts.tile([48, 3 * 16], dt.float16, tag="w2s")
    w2tap_t = consts.tile([16, 9 * 16], dt.float16, tag="w2tap")
    b1_t = consts.tile([16, 1], dt.float32, tag="b1")
    b2_t = consts.tile([16, 1], dt.float32, tag="b2")
    id_t = consts.tile([16, 16], dt.float16, tag="ident")
    inp3_t = bigs.tile([96, NIN * W], dt.float16, tag="inp3")
    x_t = bigs.tile([16, NX * W], dt.float16, tag="x")
    xT_t = bigs.tile([128, NX * 16], dt.float16, tag="xT")
    xrepM_t = bigs.tile([128, NG * 2 * XSLAB], dt.float16, tag="xrepM")
    xrepT_t = bigs.tile([48, NG * XSLAB], dt.float16, tag="xrepT")
    btM_t = bigs.tile([128, NBLK * MROW], dt.float16, tag="btM")
    btT_t = bigs.tile([48, NBLK * TROW], dt.float16, tag="btT")
    y3_t = bigs.tile([48, NY * W], dt.float16, tag="y3")
    out_t = bigs.tile([16, HS * W], dt.float32, tag="out")

    scratch_t = bigs.tile([128, 144], dt.float16, tag="scratch")
    zeros_t = bigs.tile([48, 16], dt.float16, tag="zeros")

    # ---- input DMAs, all on the sync ring in priority order: inp3
    # (conv1 critical path), weights, then the big band stream. The gpsimd
    # ring is reserved for the xTd stores + half the xrep loads; the scalar
    # ring stays clear for gelu until conv1 finishes. ----
    ih = NIN * W // 2
    nc.sync.dma_start(inp3_t[:, :ih], inp3_d[:, :ih])
    nc.sync.dma_start(w1s_t[:], w1s_d)
    nc.sync.dma_start(w1s6_t[:], w1s6_d)
    nc.sync.dma_start(b1_t[:], b1t_d)
    nc.sync.dma_start(id_t[:], ident_d)
    nc.sync.dma_start(inp3_t[:, ih:], inp3_d[:, ih:])
    nc.sync.dma_start(w2s_t[:], w2s_d)
    nc.sync.dma_start(b2_t[:], b2t_d)
    nc.sync.dma_start(w2tap_t[:], w2tap_d)
    for q in range(3):
        sl = slice(3 * q * MROW, 3 * (q + 1) * MROW)
        nc.gpsimd.dma_start(btM_t[:, sl], bandM_d[:, sl])
        st = slice(3 * q * TROW, 3 * (q + 1) * TROW)
        nc.gpsimd.dma_start(btT_t[:, st], bandT_d[:, st])
    nc.vector.memset(scratch_t[:], 0.0)
    nc.vector.memset(zeros_t[:], 0.0)

    pwarm = ps_c2.tile([16, 512], dt.float32, tag="c2")

    def warm_mm(n):
        for _ in range(n):
            nc.tensor.matmul(pwarm[:, 0:128], scratch_t[:, 0:16],
                             scratch_t[:, 16:144], start=True, stop=True,
                             skip_group_check=True)

    # y3 edge zeros: w=0 of the dx=+1 slot, w=127 of the dx=-1 slot (engine
    # ops need a 32-aligned partition base, so zero the column across all 48
    # partitions; later eviction/shift writes overwrite the rest)
    y3_v = y3_t[:].rearrange("p (r w) -> p r w", r=NY)
    nc.vector.memset(y3_v[:, :, 0:1], 0.0)
    nc.vector.memset(y3_v[:, :, W - 1:W], 0.0)

    # ---- conv1 + GELU -> x (bf16), transposes interleaved ----------------
    inp3_v = inp3_t[:].rearrange("p (r w) -> p r w", r=NIN)
    x_v = x_t[:].rearrange("p (r w) -> p r w", r=NX)

    xrep_n = 0
    xrep_engines = [nc.sync, nc.gpsimd]

    def xrep_dma(i, o, half):
        # dst slab (i, o): xrep[16i:16i+16, (g, o, y, c)] <- the DRAM copy
        # of xT, rows [8o+i+18*half, +18) for all 8 column groups. kh=8o+i.
        # (DRAM source because SBUF DMA APs can only cross partitions on
        # dim0; the (v, g)->partition 16g+v gather needs a split.)
        nonlocal xrep_n
        eng = xrep_engines[xrep_n % len(xrep_engines)]
        xrep_n += 1
        HNY = NY // 2
        ycs = slice(16 * HNY * half, 16 * HNY * (half + 1))
        if o < 2:
            dst = xrepM_t[16 * i:16 * i + 16, :].rearrange(
                "v (g o yc) -> v g o yc", g=NG, o=2)[:, :, o, ycs]
        else:
            dst = xrepT_t[16 * i:16 * i + 16, :].rearrange(
                "v (g yc) -> v g yc", g=NG)[:, :, ycs]
        r0 = 8 * o + i + HNY * half
        src = xTd_d.rearrange("(g v) f -> v g f", g=NG)[
            :, :, 16 * r0:16 * (r0 + HNY)]
        eng.dma_start(dst, src)

    # (i, o, half): xT rows [8o+i+18h, +18) must be in the DRAM xT copy
    xrep_todo = [(i, o, hf) for hf in range(2) for o in range(3)
                 for i in range(8 if o < 2 else 3)]

    def xrep_rows_needed(t):
        return 8 * t[1] + t[0] + 18 * t[2] + 18

    xrep_todo.sort(key=xrep_rows_needed)
    xtd_stores = {tb: (8 * tb, 8 * tb + 8) for tb in range(6)}

    for cb in range(NX // 4):
        psum = ps_c1.tile([16, 512], dt.float32, tag="c1")
        nc.tensor.matmul(
            psum[:], w1s6_t[:],
            inp3_v[0:96, 4 * cb: 4 * cb + 4, :],
            start=True, stop=False)
        nc.tensor.matmul(
            psum[:], w1s_t[:, 32:48],
            inp3_v[0:48, 4 * cb + 2: 4 * cb + 6, :],
            start=False, stop=True)
        nc.scalar.activation(x_t[:, 512 * cb: 512 * (cb + 1)], psum[:],
                             AFT.Gelu, bias=b1_t[:])
        if cb % 2 == 1:
            tb = cb // 2
            pt = ps_tp.tile([128, 128], dt.float16, tag="tp")
            for rr in range(8):
                nc.tensor.transpose(pt[:, 16 * rr: 16 * rr + 16],
                                    x_v[:, 8 * tb + rr, :], id_t[:])
            nc.vector.tensor_copy(xT_t[:, 128 * tb: 128 * (tb + 1)], pt[:])
            if tb in xtd_stores:
                r0, r1 = xtd_stores[tb]
                nc.scalar.dma_start(xTd_d[:, 16 * r0:16 * r1],
                                    xT_t[:, 16 * r0:16 * r1])
                rows_in_dram = r1
                # launch xrep pieces whose source rows are now in DRAM
                rest = []
                for t in xrep_todo:
                    if xrep_rows_needed(t) <= rows_in_dram:
                        xrep_dma(*t)
                    else:
                        rest.append(t)
                xrep_todo = rest
    # final xT rows to DRAM, then the remaining xrep pieces
    nc.scalar.dma_start(xTd_d[:, 16 * 48:], xT_t[:, 16 * 48:])
    for t in xrep_todo:
        xrep_dma(*t)
    xrep_todo = []

    # ---- per-pixel conv: group-blocked compact-band matmuls --------------
    btM_v = btM_t[:].rearrange("p (blk ys g o j) -> p blk ys g o j",
                               blk=NBLK, ys=4, g=NG, o=2)
    btT_v = btT_t[:].rearrange("p (blk ys g j) -> p blk ys g j",
                               blk=NBLK, ys=4, g=NG)
    def conv2_blk(c2b):
        psum = ps_c2.tile([16, 512], dt.float32, tag="c2")
        for dy in range(3):
            nc.tensor.matmul(
                psum[:],
                w2s_t[:, dy * 16:(dy + 1) * 16],
                y3_v[:, 4 * c2b + 1 + dy: 4 * c2b + 5 + dy, :],
                start=(dy == 0), stop=(dy == 2))
        nc.scalar.activation(out_t[:, 512 * c2b: 512 * (c2b + 1)],
                             psum[:], AFT.Sigmoid, bias=b2_t[:])
        nc.sync.dma_start(out_d[:, 512 * c2b: 512 * (c2b + 1)],
                          out_t[:, 512 * c2b: 512 * (c2b + 1)])

    for blk in range(NBLK):
        pp = ps_pp.tile([16, 512], dt.float32, tag="pp")
        nc.vector.memset(pp[:], 0.0)
        n_mm = 4 * NG * 3
        k = 0
        # kh octets 0,1 first: gives the xrepT/tail DMAs extra slack
        for o in (0, 1, 2):
            for ys in range(4):
                y = 4 * blk + ys
                for g in range(NG):
                    j0 = 9 if g == 0 else 0
                    j1 = 25 if g == NG - 1 else WIN
                    c0 = 128 * ys + G * g - 9
                    k += 1
                    if o < 2:
                        lhsT = xrepM_t[:, (2 * g + o) * XSLAB + 16 * y:
                                       (2 * g + o) * XSLAB + 16 * y + 16]
                        rhs = btM_v[:, blk, ys, g, o, j0:j1]
                    else:
                        lhsT = xrepT_t[:, g * XSLAB + 16 * y:
                                       g * XSLAB + 16 * y + 16]
                        rhs = btT_v[:, blk, ys, g, j0:j1]
                    nc.tensor.matmul(
                        pp[:, c0 + j0: c0 + j1], lhsT, rhs,
                        start=False, stop=(k == n_mm),
                        skip_group_check=True)
        # evict 4 y rows (f32 psum -> bf16 y3) + dx-shifted copies (skipped
        # for the last block: its conv2 uses per-tap matmuls instead)
        nc.vector.tensor_copy(y3_v[0:16, 4 * blk: 4 * blk + 4, :],
                              pp[:].rearrange("p (r w) -> p r w", r=4))
        if blk < NBLK - 1:
            nc.sync.dma_start(y3_v[16:32, 4 * blk: 4 * blk + 4, 1:W],
                                y3_v[0:16, 4 * blk: 4 * blk + 4, 0:W - 1])
            nc.sync.dma_start(y3_v[32:48, 4 * blk: 4 * blk + 4, 0:W - 1],
                                y3_v[0:16, 4 * blk: 4 * blk + 4, 1:W])
        # conv2 + sigmoid, two blocks behind (covers shift-DMA latency)
        if blk >= 2:
            conv2_blk(blk - 2)

    # last conv2 block without waiting on shift DMAs: 9 accumulating
    # matmuls (one per conv tap) reading the unshifted y3 with w-sliced APs
    c2b = NBLK - 2
    psum = ps_c2.tile([16, 512], dt.float32, tag="c2")
    first = True
    for dy in range(3):
        for gi, dx in enumerate((1, 0, 2)):
            lhsT = w2tap_t[:, (3 * dy + dx) * 16:(3 * dy + dx) * 16 + 16]
            if dx == 1:
                src, dst = slice(0, W), slice(0, W)
            elif dx == 0:
                src, dst = slice(0, W - 1), slice(1, W)
            else:
                src, dst = slice(1, W), slice(0, W - 1)
            nc.tensor.matmul(
                psum[:].rearrange("p (r w) -> p r w", r=4)[:, :, dst],
                lhsT,
                y3_v[0:16, 4 * c2b + 1 + dy: 4 * c2b + 5 + dy, src],
                start=first, stop=(dy == 2 and gi == 2),
                skip_group_check=True)
            first = False
    nc.scalar.activation(out_t[:, 512 * c2b: 512 * (c2b + 1)],
                         psum[:], AFT.Sigmoid, bias=b2_t[:])
    nc.scalar.dma_start(out_d[:, 512 * c2b: 512 * (c2b + 1)],
                        out_t[:, 512 * c2b: 512 * (c2b + 1)])
    if dbg is not None:
        nc.sync.dma_start(dbg["xT"], xT_t[:])
        nc.sync.dma_start(dbg["xrepM"], xrepM_t[:])
        nc.sync.dma_start(dbg["xrepT"], xrepT_t[:])
        nc.sync.dma_start(dbg["y3"], y3_t[:])


_NC_CACHE = None
LAST = {}


def _get_nc():
    global _NC_CACHE
    if _NC_CACHE is None:
        _NC_CACHE = _build_program()
    return _NC_CACHE


def kernel(input, kernel, w1, b1, w2, b2, _trace=False, _tmpdir=None):
    in_maps = _host_prepare(input, kernel, w1, b1, w2, b2)
    nc = _get_nc()
    res = run_bass_kernel_spmd(nc, in_maps, core_ids=list(range(NCORES)),
                               trace=_trace, tmpdir=_tmpdir)
    out = np.zeros((B, C, H, W), np.float32)
    for cid in range(NCORES):
        b = cid // 4
        h0 = 32 * (cid % 4)
        out[b, :, h0:h0 + HS, :] = res.results[cid]["out"].reshape(16, HS, W)
    LAST["exec_ns"] = res.exec_time_ns
    LAST["trace"] = res.instructions_and_trace
    return out


# revision 58
# speedup vs baseline: 1.0273x; 1.0273x over previous
"""Trainium2 Bass kernel for nn_Kernel_Conv (conv3x3+GELU -> per-pixel 19x19
conv -> conv3x3+sigmoid), SPMD over 8 NeuronCores.

Sharding: 8 cores = 2 batches x 4 H-slices (32 output rows each). All inputs
are host-preprocessed into per-core slabs (bf16) so the device program is
identical on every core.

Per-pixel conv strategy (v2, group-blocked): for output row y, split the 128
x-columns into 8 groups of 16. For group g, the contribution to out[c, w]
for w in the 34-wide window [16g-9, 16g+25) is a matmul contracting over
(kh, v): lhsT[(kh,v), c] = x[y+kh-9, 16g+v, c] (an on-chip replicated
transpose of x) and rhs[(kh,v), j] = ker[kh*19 + (v-j+18), y, 16g-9+j]
(host-skewed compact band, zero outside the valid diagonal). kh is packed in
octets of 8 (8*16 = 128 contraction partitions), so each (y, g) takes 3
matmuls (kh 0-7, 8-15, 16-18) accumulating into a per-y psum row window.
Overlapping windows accumulate onto a DVE-zeroed psum (start=False).

The band reaches SBUF via contiguous >=4KB-per-partition DMA descriptors (the
host bakes the skew + zeros), eliminating the 68-byte scatter packets that
made the previous version DMA-bound.
"""

import sys
import types

for _p in ("/opt/trn_rl_repo",):
    if _p not in sys.path:
        sys.path.insert(0, _p)

import numpy as np
import ml_dtypes
from contextlib import ExitStack

# Register the NTFF profile hook shim (harmless if tracing is never used)
try:
    import antenv  # noqa: F401
    if "antenv.axon_hooks" not in sys.modules:
        if "/root/.axon_site" not in sys.path:
            sys.path.insert(0, "/root/.axon_site")
        from trn_agent_boot.trn_boot import _ntff_profile_via_ctypes
        _hook = _ntff_profile_via_ctypes("/opt/axon/libaxon_pjrt.so")
        _mod = types.ModuleType("antenv.axon_hooks")
        _mod.get_axon_ntff_profile_hook = lambda: _hook
        sys.modules["antenv.axon_hooks"] = _mod
        antenv.axon_hooks = _mod
except Exception:
    pass

import bass_rust
import concourse.bass as bass
import concourse.tile as tile
from concourse import bacc, mybir
from concourse.bass_utils import run_bass_kernel_spmd

BF16 = np.float16

# ---------------- problem constants (hardcoded per the harness contract) ----
B, C, H, W = 2, 16, 128, 128
KK = 19            # per-pixel kernel size
NCORES = 8
HS = 32            # output rows per core
NY = 36            # y rows per core: [h0-2, h0+34)
NX = 56            # x (conv1 out) rows per core: [h0-11, h0+45)
NIN = 58           # input rows per core: [h0-12, h0+46)
G = 16             # x-column group size for the per-pixel matmuls
NG = W // G        # 8 groups
WIN = G + KK - 1   # 34: output-column window per group
NBLK = NY // 4     # 9 per-pixel blocks of 4 y rows
MROW = 2 * NG * 4 * WIN       # 2176: main band els per (partition, block)
TROW = NG * 4 * WIN           # 1088: tail band els per (partition, block)
XSLAB = NY * 16               # 576: (y, c) els per xrep slab


def _host_prepare(input, kernel, w1, b1, w2, b2):
    """Build the per-core input maps (all numpy, bf16 except biases/ident)."""
    inp = np.asarray(input, np.float32)
    ker = np.asarray(kernel, np.float32)

    # input, zero-padded: rows [-12, 140), cols [-1, 129)
    inp_pad = np.zeros((B, C, H + 26, W + 2), np.float32)
    inp_pad[:, :, 12:12 + H, 1:1 + W] = inp

    # conv weights as lhsT[(dx,c), o] per dy: [3, 48, 16]
    def wstack(wmat, order=(0, 1, 2)):
        ws = np.zeros((3, 48, 16), np.float32)
        for dy in range(3):
            for gi, dx in enumerate(order):
                ws[dy, gi * 16:gi * 16 + 16, :] = wmat[:, :, dy, dx].T
        return np.ascontiguousarray(ws.transpose(1, 0, 2)).astype(BF16)

    w1s = wstack(np.asarray(w1, np.float32))
    w2s = wstack(np.asarray(w2, np.float32), order=(1, 0, 2))
    # per-tap lhsT for the final conv2 block: [c', (dy, dx, o)]
    w2tap = np.ascontiguousarray(
        np.asarray(w2, np.float32).transpose(1, 2, 3, 0).reshape(16, 144)
    ).astype(BF16)
    b1t = np.asarray(b1, np.float32).reshape(16, 1).copy()
    b2t = np.asarray(b2, np.float32).reshape(16, 1).copy()
    ident = np.eye(16, dtype=BF16)

    # ---- compact skewed band ---------------------------------------------
    # main: F[(i8,v16), blk9, ys4, g8, o2, j34], kh = 8o+i
    # tail: T[(i3,v16), blk9, ys4, g8, j34],     kh = 16+i
    # value = ker[b, kh*19 + (v-j+18), y, 16g-9+j] masked to validity, where
    # y = h0-2+4blk+ys; mask: kw in [0,19), w in [0,W), y in [0,H),
    # x-row y+kh-9 in [0,H).
    def band(kh_arr, h0, b):
        # kh_arr: [NI] kernel-row indices; returns [NI*16, NBLK*rest]
        NI = kh_arr.shape[0]
        i_ = kh_arr.reshape(NI, 1, 1, 1, 1, 1, 1)
        v_ = np.arange(G).reshape(1, G, 1, 1, 1, 1, 1)
        bl = np.arange(NBLK).reshape(1, 1, NBLK, 1, 1, 1, 1)
        ys = np.arange(4).reshape(1, 1, 1, 4, 1, 1, 1)
        g_ = np.arange(NG).reshape(1, 1, 1, 1, NG, 1, 1)
        j_ = np.arange(WIN).reshape(1, 1, 1, 1, 1, 1, WIN)
        kh = i_
        y = h0 - 2 + 4 * bl + ys
        kw = v_ - j_ + 18
        w = G * g_ - 9 + j_
        xr = y + kh - 9
        valid = ((kw >= 0) & (kw < KK) & (w >= 0) & (w < W)
                 & (y >= 0) & (y < H) & (xr >= 0) & (xr < H))
        kwc = np.clip(kw, 0, KK - 1)
        wc = np.clip(w, 0, W - 1)
        yc = np.clip(y, 0, H - 1)
        F = ker[b, kh * KK + kwc, yc, wc] * valid
        # [NI, v, blk, ys, g, 1, j] -> [NI*16, blk*(ys*g*1*j)]
        F = np.broadcast_to(F, (NI, G, NBLK, 4, NG, 1, WIN))
        return np.ascontiguousarray(
            F.reshape(NI * G, NBLK * 4 * NG * WIN)).astype(BF16)

    in_maps = []
    for cid in range(NCORES):
        b = cid // 4
        h0 = 32 * (cid % 4)

        # dx-tripled input slab [48, NIN, 128]
        inp3 = np.zeros((48, NIN, W), np.float32)
        rows = inp_pad[b, :, 12 + h0 - 12: 12 + h0 - 12 + NIN, :]
        for dx in range(3):
            inp3[dx * 16:dx * 16 + 16, :, :] = rows[:, :, dx:dx + W]
        inp3 = inp3.astype(BF16)

        # main band: interleave octets o=0,1 -> [(i,v), blk, ys, g, o, j]
        bm0 = band(np.arange(0, 8), h0, b).reshape(128, NBLK, 4, NG, 1, WIN)
        bm1 = band(np.arange(8, 16), h0, b).reshape(128, NBLK, 4, NG, 1, WIN)
        bandM = np.concatenate([bm0, bm1], axis=4)  # o dim
        bandM = np.ascontiguousarray(
            bandM.reshape(128, NBLK * MROW)).astype(BF16)
        bandT = band(np.arange(16, 19), h0, b)      # [48, NBLK*TROW]

        in_maps.append({
            "inp3": np.ascontiguousarray(inp3.reshape(48, NIN * W)),
            "bandM": bandM,
            "bandT": bandT,
            "w1s": np.ascontiguousarray(w1s.reshape(48, 3 * 16)),
            "w2s": np.ascontiguousarray(w2s.reshape(48, 3 * 16)),
            "w2tap": w2tap,
            "b1t": b1t,
            "b2t": b2t,
            "ident": ident,
        })
    return in_maps


DEBUG_DUMP = False


def _build_program():
    nc = bacc.Bacc("TRN2", target_bir_lowering=False, debug=False,
                   num_devices=NCORES)
    dt = mybir.dt

    inp3_d = nc.dram_tensor("inp3", [48, NIN * W], dt.float16,
                            kind="ExternalInput").ap()
    bandM_d = nc.dram_tensor("bandM", [128, NBLK * MROW], dt.float16,
                             kind="ExternalInput").ap()
    bandT_d = nc.dram_tensor("bandT", [48, NBLK * TROW], dt.float16,
                             kind="ExternalInput").ap()
    w1s_d = nc.dram_tensor("w1s", [48, 3 * 16], dt.float16,
                           kind="ExternalInput").ap()
    w2s_d = nc.dram_tensor("w2s", [48, 3 * 16], dt.float16,
                           kind="ExternalInput").ap()
    w2tap_d = nc.dram_tensor("w2tap", [16, 9 * 16], dt.float16,
                             kind="ExternalInput").ap()
    b1t_d = nc.dram_tensor("b1t", [16, 1], dt.float32, kind="ExternalInput").ap()
    b2t_d = nc.dram_tensor("b2t", [16, 1], dt.float32, kind="ExternalInput").ap()
    ident_d = nc.dram_tensor("ident", [16, 16], dt.float16,
                             kind="ExternalInput").ap()
    out_d = nc.dram_tensor("out", [16, HS * W], dt.float32,
                           kind="ExternalOutput").ap()
    xTd_d = nc.dram_tensor("xTd", [128, NX * 16], dt.float16,
                           kind="Internal").ap()
    dbg = None
    if DEBUG_DUMP:
        dbg = {
            "xT": nc.dram_tensor("dbg_xT", [128, NX * 16], dt.float16,
                                 kind="ExternalOutput").ap(),
            "xrepM": nc.dram_tensor("dbg_xrepM", [128, NG * 2 * XSLAB],
                                    dt.float16, kind="ExternalOutput").ap(),
            "xrepT": nc.dram_tensor("dbg_xrepT", [48, NG * XSLAB],
                                    dt.float16, kind="ExternalOutput").ap(),
            "y3": nc.dram_tensor("dbg_y3", [48, NY * W], dt.float16,
                                 kind="ExternalOutput").ap(),
        }

    with tile.TileContext(nc) as tc:
        with ExitStack() as ctx:
            _body(ctx, tc, inp3_d, bandM_d, bandT_d, w1s_d, w2s_d,
                  w2tap_d, b1t_d, b2t_d, ident_d, out_d, xTd_d, dbg)
    nc.compile()
    return nc


def _body(ctx, tc, inp3_d, bandM_d, bandT_d, w1s_d, w2s_d, w2tap_d,
          b1t_d, b2t_d, ident_d, out_d, xTd_d, dbg=None):
    nc = tc.nc
    dt = mybir.dt
    AFT = mybir.ActivationFunctionType

    consts = ctx.enter_context(tc.tile_pool(name="consts", bufs=1))
    bigs = ctx.enter_context(tc.tile_pool(name="bigs", bufs=1))
    ps_c1 = ctx.enter_context(tc.tile_pool(name="ps_c1", bufs=2, space="PSUM"))
    ps_tp = ctx.enter_context(tc.tile_pool(name="ps_tp", bufs=2, space="PSUM"))
    ps_pp = ctx.enter_context(tc.tile_pool(name="ps_pp", bufs=2, space="PSUM"))
    ps_c2 = ctx.enter_context(tc.tile_pool(name="ps_c2", bufs=2, space="PSUM"))

    # ---- persistent SBUF tiles -------------------------------------------
    w1s_t = consts.tile([48, 3 * 16], dt.float16, tag="w1s")
    w2s_t = consts.tile([48, 3 * 16], dt.float16, tag="w2s")
    w2tap_t = consts.tile([16, 9 * 16], dt.float16, tag="w2tap")
    b1_t = consts.tile([16, 1], dt.float32, tag="b1")
    b2_t = consts.tile([16, 1], dt.float32, tag="b2")
    id_t = consts.tile([16, 16], dt.float16, tag="ident")
    inp3_t = bigs.tile([48, NIN * W], dt.float16, tag="inp3")
    x_t = bigs.tile([16, NX * W], dt.float16, tag="x")
    xT_t = bigs.tile([128, NX * 16], dt.float16, tag="xT")
    xrepM_t = bigs.tile([128, NG * 2 * XSLAB], dt.float16, tag="xrepM")
    xrepT_t = bigs.tile([48, NG * XSLAB], dt.float16, tag="xrepT")
    btM_t = bigs.tile([128, NBLK * MROW], dt.float16, tag="btM")
    btT_t = bigs.tile([48, NBLK * TROW], dt.float16, tag="btT")
    y3_t = bigs.tile([48, NY * W], dt.float16, tag="y3")
    out_t = bigs.tile([16, HS * W], dt.float32, tag="out")

    scratch_t = bigs.tile([128, 144], dt.float16, tag="scratch")
    zeros_t = bigs.tile([48, 16], dt.float16, tag="zeros")

    # ---- input DMAs, all on the sync ring in priority order: inp3
    # (conv1 critical path), weights, then the big band stream. The gpsimd
    # ring is reserved for the xTd stores + half the xrep loads; the scalar
    # ring stays clear for gelu until conv1 finishes. ----
    ih = NIN * W // 2
    nc.sync.dma_start(inp3_t[:, :ih], inp3_d[:, :ih])
    nc.sync.dma_start(w1s_t[:], w1s_d)
    nc.sync.dma_start(b1_t[:], b1t_d)
    nc.sync.dma_start(id_t[:], ident_d)
    nc.sync.dma_start(inp3_t[:, ih:], inp3_d[:, ih:])
    nc.sync.dma_start(w2s_t[:], w2s_d)
    nc.sync.dma_start(b2_t[:], b2t_d)
    nc.sync.dma_start(w2tap_t[:], w2tap_d)
    for q in range(3):
        sl = slice(3 * q * MROW, 3 * (q + 1) * MROW)
        nc.gpsimd.dma_start(btM_t[:, sl], bandM_d[:, sl])
        st = slice(3 * q * TROW, 3 * (q + 1) * TROW)
        nc.gpsimd.dma_start(btT_t[:, st], bandT_d[:, st])
    nc.vector.memset(scratch_t[:], 0.0)
    nc.vector.memset(zeros_t[:], 0.0)

    pwarm = ps_c2.tile([16, 512], dt.float32, tag="c2")

    def warm_mm(n):
        for _ in range(n):
            nc.tensor.matmul(pwarm[:, 0:128], scratch_t[:, 0:16],
                             scratch_t[:, 16:144], start=True, stop=True,
                             skip_group_check=True)

    # y3 edge zeros: w=0 of the dx=+1 slot, w=127 of the dx=-1 slot (engine
    # ops need a 32-aligned partition base, so zero the column across all 48
    # partitions; later eviction/shift writes overwrite the rest)
    y3_v = y3_t[:].rearrange("p (r w) -> p r w", r=NY)
    nc.vector.memset(y3_v[:, :, 0:1], 0.0)
    nc.vector.memset(y3_v[:, :, W - 1:W], 0.0)

    # ---- conv1 + GELU -> x (bf16), transposes interleaved ----------------
    inp3_v = inp3_t[:].rearrange("p (r w) -> p r w", r=NIN)
    x_v = x_t[:].rearrange("p (r w) -> p r w", r=NX)

    xrep_n = 0
    xrep_engines = [nc.sync, nc.gpsimd]

    def xrep_dma(i, o, half):
        # dst slab (i, o): xrep[16i:16i+16, (g, o, y, c)] <- the DRAM copy
        # of xT, rows [8o+i+18*half, +18) for all 8 column groups. kh=8o+i.
        # (DRAM source because SBUF DMA APs can only cross partitions on
        # dim0; the (v, g)->partition 16g+v gather needs a split.)
        nonlocal xrep_n
        eng = xrep_engines[xrep_n % len(xrep_engines)]
        xrep_n += 1
        HNY = NY // 2
        ycs = slice(16 * HNY * half, 16 * HNY * (half + 1))
        if o < 2:
            dst = xrepM_t[16 * i:16 * i + 16, :].rearrange(
                "v (g o yc) -> v g o yc", g=NG, o=2)[:, :, o, ycs]
        else:
            dst = xrepT_t[16 * i:16 * i + 16, :].rearrange(
                "v (g yc) -> v g yc", g=NG)[:, :, ycs]
        r0 = 8 * o + i + HNY * half
        src = xTd_d.rearrange("(g v) f -> v g f", g=NG)[
            :, :, 16 * r0:16 * (r0 + HNY)]
        eng.dma_start(dst, src)

    # (i, o, half): xT rows [8o+i+18h, +18) must be in the DRAM xT copy
    xrep_todo = [(i, o, hf) for hf in range(2) for o in range(3)
                 for i in range(8 if o < 2 else 3)]

    def xrep_rows_needed(t):
        return 8 * t[1] + t[0] + 18 * t[2] + 18

    xrep_todo.sort(key=xrep_rows_needed)
    xtd_stores = {tb: (8 * tb, 8 * tb + 8) for tb in range(6)}

    for cb in range(NX // 4):
        psum = ps_c1.tile([16, 512], dt.float32, tag="c1")
        for dy in range(3):
            nc.tensor.matmul(
                psum[:],
                w1s_t[:, dy * 16:(dy + 1) * 16],
                inp3_v[:, 4 * cb + dy: 4 * cb + dy + 4, :],
                start=(dy == 0), stop=(dy == 2))
        nc.scalar.activation(x_t[:, 512 * cb: 512 * (cb + 1)], psum[:],
                             AFT.Gelu, bias=b1_t[:])
        if cb % 2 == 1:
            tb = cb // 2
            pt = ps_tp.tile([128, 128], dt.float16, tag="tp")
            for rr in range(8):
                nc.tensor.transpose(pt[:, 16 * rr: 16 * rr + 16],
                                    x_v[:, 8 * tb + rr, :], id_t[:])
            nc.vector.tensor_copy(xT_t[:, 128 * tb: 128 * (tb + 1)], pt[:])
            if tb in xtd_stores:
                r0, r1 = xtd_stores[tb]
                nc.scalar.dma_start(xTd_d[:, 16 * r0:16 * r1],
                                    xT_t[:, 16 * r0:16 * r1])
                rows_in_dram = r1
                # launch xrep pieces whose source rows are now in DRAM
                rest = []
                for t in xrep_todo:
                    if xrep_rows_needed(t) <= rows_in_dram:
                        xrep_dma(*t)
                    else:
                        rest.append(t)
                xrep_todo = rest
    # final xT rows to DRAM, then the remaining xrep pieces
    nc.scalar.dma_start(xTd_d[:, 16 * 48:], xT_t[:, 16 * 48:])
    for t in xrep_todo:
        xrep_dma(*t)
    xrep_todo = []

    # ---- per-pixel conv: group-blocked compact-band matmuls --------------
    btM_v = btM_t[:].rearrange("p (blk ys g o j) -> p blk ys g o j",
                               blk=NBLK, ys=4, g=NG, o=2)
    btT_v = btT_t[:].rearrange("p (blk ys g j) -> p blk ys g j",
                               blk=NBLK, ys=4, g=NG)
    def conv2_blk(c2b):
        psum = ps_c2.tile([16, 512], dt.float32, tag="c2")
        for dy in range(3):
            nc.tensor.matmul(
                psum[:],
                w2s_t[:, dy * 16:(dy + 1) * 16],
                y3_v[:, 4 * c2b + 1 + dy: 4 * c2b + 5 + dy, :],
                start=(dy == 0), stop=(dy == 2))
        nc.scalar.activation(out_t[:, 512 * c2b: 512 * (c2b + 1)],
                             psum[:], AFT.Sigmoid, bias=b2_t[:])
        nc.sync.dma_start(out_d[:, 512 * c2b: 512 * (c2b + 1)],
                          out_t[:, 512 * c2b: 512 * (c2b + 1)])

    for blk in range(NBLK):
        pp = ps_pp.tile([16, 512], dt.float32, tag="pp")
        nc.vector.memset(pp[:], 0.0)
        n_mm = 4 * NG * 3
        k = 0
        # kh octets 0,1 first: gives the xrepT/tail DMAs extra slack
        for o in (0, 1, 2):
            for ys in range(4):
                y = 4 * blk + ys
                for g in range(NG):
                    j0 = 9 if g == 0 else 0
                    j1 = 25 if g == NG - 1 else WIN
                    c0 = 128 * ys + G * g - 9
                    k += 1
                    if o < 2:
                        lhsT = xrepM_t[:, (2 * g + o) * XSLAB + 16 * y:
                                       (2 * g + o) * XSLAB + 16 * y + 16]
                        rhs = btM_v[:, blk, ys, g, o, j0:j1]
                    else:
                        lhsT = xrepT_t[:, g * XSLAB + 16 * y:
                                       g * XSLAB + 16 * y + 16]
                        rhs = btT_v[:, blk, ys, g, j0:j1]
                    nc.tensor.matmul(
                        pp[:, c0 + j0: c0 + j1], lhsT, rhs,
                        start=False, stop=(k == n_mm),
                        skip_group_check=True)
        # evict 4 y rows (f32 psum -> bf16 y3) + dx-shifted copies (skipped
        # for the last block: its conv2 uses per-tap matmuls instead)
        nc.vector.tensor_copy(y3_v[0:16, 4 * blk: 4 * blk + 4, :],
                              pp[:].rearrange("p (r w) -> p r w", r=4))
        if blk < NBLK - 1:
            nc.sync.dma_start(y3_v[16:32, 4 * blk: 4 * blk + 4, 1:W],
                                y3_v[0:16, 4 * blk: 4 * blk + 4, 0:W - 1])
            nc.sync.dma_start(y3_v[32:48, 4 * blk: 4 * blk + 4, 0:W - 1],
                                y3_v[0:16, 4 * blk: 4 * blk + 4, 1:W])
        # conv2 + sigmoid, two blocks behind (covers shift-DMA latency)
        if blk >= 2:
            conv2_blk(blk - 2)

    # last conv2 block without waiting on shift DMAs: 9 accumulating
    # matmuls (one per conv tap) reading the unshifted y3 with w-sliced APs
    c2b = NBLK - 2
    psum = ps_c2.tile([16, 512], dt.float32, tag="c2")
    first = True
    for dy in range(3):
        for gi, dx in enumerate((1, 0, 2)):
            lhsT = w2tap_t[:, (3 * dy + dx) * 16:(3 * dy + dx) * 16 + 16]
            if dx == 1:
                src, dst = slice(0, W), slice(0, W)
            elif dx == 0:
                src, dst = slice(0, W - 1), slice(1, W)
            else:
                src, dst = slice(1, W), slice(0, W - 1)
            nc.tensor.matmul(
                psum[:].rearrange("p (r w) -> p r w", r=4)[:, :, dst],
                lhsT,
                y3_v[0:16, 4 * c2b + 1 + dy: 4 * c2b + 5 + dy, src],
                start=first, stop=(dy == 2 and gi == 2),
                skip_group_check=True)
            first = False
    nc.scalar.activation(out_t[:, 512 * c2b: 512 * (c2b + 1)],
                         psum[:], AFT.Sigmoid, bias=b2_t[:])
    nc.scalar.dma_start(out_d[:, 512 * c2b: 512 * (c2b + 1)],
                        out_t[:, 512 * c2b: 512 * (c2b + 1)])
    if dbg is not None:
        nc.sync.dma_start(dbg["xT"], xT_t[:])
        nc.sync.dma_start(dbg["xrepM"], xrepM_t[:])
        nc.sync.dma_start(dbg["xrepT"], xrepT_t[:])
        nc.sync.dma_start(dbg["y3"], y3_t[:])


_NC_CACHE = None
LAST = {}


def _get_nc():
    global _NC_CACHE
    if _NC_CACHE is None:
        _NC_CACHE = _build_program()
    return _NC_CACHE


def kernel(input, kernel, w1, b1, w2, b2, _trace=False, _tmpdir=None):
    in_maps = _host_prepare(input, kernel, w1, b1, w2, b2)
    nc = _get_nc()
    res = run_bass_kernel_spmd(nc, in_maps, core_ids=list(range(NCORES)),
                               trace=_trace, tmpdir=_tmpdir)
    out = np.zeros((B, C, H, W), np.float32)
    for cid in range(NCORES):
        b = cid // 4
        h0 = 32 * (cid % 4)
        out[b, :, h0:h0 + HS, :] = res.results[cid]["out"].reshape(16, HS, W)
    LAST["exec_ns"] = res.exec_time_ns
    LAST["trace"] = res.instructions_and_trace
    return out
